# revision 9
# baseline (speedup 1.0000x reference)
"""Trainium2 Bass kernel: 7x7 valid 2D cross-correlation of an 8192x8192
fp32 image plus scalar bias, row-sharded across 8 NeuronCores.

Formulation (per core): the y-direction 7-tap convolution for a fixed kernel
column dx is a banded matmul: out_dx[y, x] = sum_r A_dx[r, y] * X[r, x] with
A_dx[r, y] = K[r - y, dx].  The full conv accumulates the 7 dx terms in PSUM
with the moving operand (image columns) shifted by dx.  Matmuls run in bf16
(inputs bf16, fp32 PSUM accumulate); the banded weight blocks are padded to
128 columns so the compiler's fast-weight-load path engages.

Work distribution: 8186 output rows = 68 bands of <=122 rows.  Each core gets
8 full bands (rows 976*i .. 976*i+976) plus HALF of one of bands 64..67
(8 column tiles), i.e. 136 (band, col-tile) units/core instead of 9 full
bands = 144 — the PE-time quantum is a 512-column matmul pass, so the old
layout wasted 8 units/core on a mostly-empty 9th band.  The half-band is
processed FIRST: its input is only ~1 MB, so the PE starts as soon as the
DMA rings come up instead of waiting for a full 2.1 MB slab.  Output is
stored per 1024-column pair tile immediately after its PSUM drain, so the
kernel tail after the last matmul is one small store, not a 2 MB band store.
"""

import numpy as np
import ml_dtypes

import concourse.bass as bass
import concourse.mybir as mybir
from concourse.tile import TileContext
from concourse.bass_utils import run_bass_kernel_spmd

H = W = 8192
KH = KW = 7
OH = OW = H - KH + 1          # 8186
N_CORES = 8
BAND_IN = 128                 # input rows per matmul band (partition dim)
BAND_OUT = BAND_IN - KH + 1   # 122 output rows per band
APAD = 128                    # A block columns (padded from BAND_OUT for FWL)
COL_TILE = 512                # moving-operand free dim (one PSUM bank, fp32)
F32 = mybir.dt.float32
BF16 = mybir.dt.bfloat16

MAIN_BANDS = 8                # full bands per core
MAIN_OUT = MAIN_BANDS * BAND_OUT      # 976
MAIN_IN = MAIN_OUT + KH - 1           # 982
HALF_TILES = 8                # col tiles in the half band
HALF_OUT_COLS = HALF_TILES * COL_TILE # 4096
HALF_IN_COLS = HALF_OUT_COLS + 8      # 4104 (6-col halo, padded to 8)

# Results object of the most recent hardware run (for test harnesses).
LAST_RESULTS = None


def _split_multi_waits(nc):
    """Walrus in this toolchain accepts at most ONE sync-wait per
    instruction; Tile's scheduler may attach several.  Hoist the extras onto
    single-wait InstEventSemaphore instructions inserted just before, on the
    same engine stream (a sequence of waits = AND of the conditions)."""
    uid = 0
    for fn in nc.m.functions:
        for blk in fn.blocks:
            new_list = []
            for inst in blk.instructions:
                si = getattr(inst, "sync_info", None)
                if si is not None and si.on_wait and len(si.on_wait) > 1:
                    waits = list(si.on_wait)
                    for w in waits[:-1]:
                        ev = mybir.InstEventSemaphore(
                            name=f"wait_split_{uid}",
                            ins=[],
                            outs=[],
                            sync_info=mybir.SyncInfo(on_wait=[w], on_update=[]),
                        )
                        uid += 1
                        ev.engine = inst.engine
                        new_list.append(ev)
                    si.on_wait = [waits[-1]]
                new_list.append(inst)
            blk.instructions[:] = new_list


def _build_nc(bias_val):
    nc = bass.Bass()
    Xm = nc.declare_dram_parameter("Xm", [MAIN_IN, W], BF16, isOutput=False)
    Xh = nc.declare_dram_parameter("Xh", [BAND_IN, HALF_IN_COLS], BF16, isOutput=False)
    A = nc.declare_dram_parameter("A", [BAND_IN, KW * APAD], BF16, isOutput=False)
    Om = nc.declare_dram_parameter("Om", [MAIN_OUT, OW], BF16, isOutput=True)
    Oh = nc.declare_dram_parameter("Oh", [BAND_OUT, HALF_OUT_COLS], BF16, isOutput=True)

    with TileContext(nc) as tc:
        with (
            tc.tile_pool(name="const", bufs=1) as cpool,
            tc.tile_pool(name="hx", bufs=1) as hxpool,
            tc.tile_pool(name="x", bufs=4) as xpool,
            tc.tile_pool(name="o", bufs=3) as opool,
            tc.tile_pool(name="ps", bufs=8, space="PSUM") as pspool,
        ):
            # Startup: the DMA rings come up staggered; split the small gating
            # loads (A, half-band input) across all four issuing queues so the
            # first matmul is gated on ~1.3 MB of 4-way-parallel traffic, not a
            # serialized 2.1 MB slab.
            # SWDGE (gpsimd) distributes one DMA's descriptors across all 16
            # SDMA engines; an HWDGE (sync/scalar) DMA serializes onto ONE
            # engine (~13 GB/s) but concurrent HWDGE DMAs run on different
            # engines in parallel.  So: small/latency-critical loads go on
            # gpsimd; anything on sync/scalar is split into many small-row
            # chunks issued back-to-back.
            a_tile = cpool.tile([BAND_IN, KW * APAD], BF16)
            nc.gpsimd.dma_start(out=a_tile[:, :], in_=A[:, :])

            # Half-band input in two column-halves: the first 2 col tiles only
            # gate on the first half.
            hx_a = hxpool.tile([BAND_IN, 2056], BF16, tag="hxa")
            hx_b = hxpool.tile([BAND_IN, HALF_IN_COLS - 2048], BF16, tag="hxb")
            nc.gpsimd.dma_start(out=hx_a[:, :], in_=Xh[:, 0:2056])

            x_tiles = {}

            def issue_load(bi, startup=False):
                if bi >= MAIN_BANDS:
                    return
                r0 = bi * BAND_OUT
                xt = xpool.tile([BAND_IN, W], BF16, tag="x")
                if startup:
                    # 8-row mini-DMAs on the HWDGE queues run concurrently on
                    # separate SDMA engines; gpsimd takes the last quarter.
                    for k in range(6):
                        nc.sync.dma_start(
                            out=xt[8 * k : 8 * k + 8, :],
                            in_=Xm[r0 + 8 * k : r0 + 8 * k + 8, :],
                        )
                    for k in range(6, 12):
                        nc.scalar.dma_start(
                            out=xt[8 * k : 8 * k + 8, :],
                            in_=Xm[r0 + 8 * k : r0 + 8 * k + 8, :],
                        )
                    nc.gpsimd.dma_start(out=xt[96:128, :], in_=Xm[r0 + 96 : r0 + 128, :])
                else:
                    nc.gpsimd.dma_start(out=xt[0:64, :], in_=Xm[r0 : r0 + 64, :])
                    nc.gpsimd.dma_start(out=xt[64:128, :], in_=Xm[r0 + 64 : r0 + 128, :])
                x_tiles[bi] = xt

            # Prefetch the first main bands behind the half-band gating loads.
            issue_load(0, startup=True)
            nc.gpsimd.dma_start(out=hx_b[:, :], in_=Xh[:, 2048:HALF_IN_COLS])
            issue_load(1)
            issue_load(2)

            def conv_tile(x_tile, x0, w, o_tile, c0):
                """7 accumulating matmuls into a PSUM bank, drain to o_tile."""
                ps = pspool.tile([APAD, COL_TILE], F32)
                for dx in range(KW):
                    nc.tensor.matmul(
                        ps[:, :w],
                        lhsT=a_tile[:, dx * APAD : (dx + 1) * APAD],
                        rhs=x_tile[:, x0 + dx : x0 + dx + w],
                        start=(dx == 0),
                        stop=(dx == KW - 1),
                    )
                nc.vector.tensor_scalar_add(
                    o_tile[:, c0 : c0 + w], ps[:BAND_OUT, :w], float(bias_val)
                )

            # --- first two half-band col tiles: tiny gate (A + hx_a) so the
            # PE starts while the first main band is still loading.
            o_half_a = opool.tile([BAND_OUT, 2 * COL_TILE], BF16, tag="oha")
            conv_tile(hx_a, 0, COL_TILE, o_half_a, 0)
            conv_tile(hx_a, COL_TILE, COL_TILE, o_half_a, COL_TILE)
            # 1024-col rows are only 2 KB packets: keep this store on gpsimd
            # (SWDGE spreads descriptors), split for overlap.
            nc.gpsimd.dma_start(out=Oh[0:61, 0:1024], in_=o_half_a[0:61, :])
            nc.gpsimd.dma_start(out=Oh[61:BAND_OUT, 0:1024], in_=o_half_a[61:BAND_OUT, :])

            # --- main bands.  Per-band wide o_tile, stored as full-width row
            # chunks (16 KB packets); HWDGE shares are ~8-row mini-DMAs so
            # several are in flight on each queue.
            for bi in range(MAIN_BANDS):
                issue_load(bi + 3)
                x_tile = x_tiles.pop(bi)
                o_tile = opool.tile([BAND_OUT, OW], BF16, tag="om")
                for j in range(16):
                    x0 = j * COL_TILE
                    w = min(COL_TILE, OW - x0)
                    conv_tile(x_tile, x0, w, o_tile, x0)
                s = bi * BAND_OUT
                bounds = [(BAND_OUT * k) // 16 for k in range(17)]
                engs = (nc.gpsimd, nc.sync, nc.gpsimd, nc.scalar) * 4
                for k in range(16):
                    p0, p1 = bounds[k], bounds[k + 1]
                    engs[k].dma_start(
                        out=Om[s + p0 : s + p1, :], in_=o_tile[p0:p1, :]
                    )

            # --- remaining six half-band col tiles last: their input has been
            # resident since startup and the tail store is only 0.75 MB.
            o_half_b = opool.tile([BAND_OUT, 6 * COL_TILE], BF16, tag="ohb")
            for j in range(2, 4):
                conv_tile(hx_a, j * COL_TILE, COL_TILE, o_half_b, (j - 2) * COL_TILE)
            for j in range(4, HALF_TILES):
                conv_tile(hx_b, j * COL_TILE - 2048, COL_TILE, o_half_b, (j - 2) * COL_TILE)
            bounds = [(BAND_OUT * k) // 12 for k in range(13)]
            engs = (nc.gpsimd, nc.sync, nc.gpsimd, nc.scalar) * 3
            for k in range(12):
                p0, p1 = bounds[k], bounds[k + 1]
                engs[k].dma_start(
                    out=Oh[p0:p1, 1024:HALF_OUT_COLS], in_=o_half_b[p0:p1, :]
                )

    _split_multi_waits(nc)
    return nc


def _make_A(K):
    A = np.zeros((BAND_IN, KW * APAD), np.float32)
    for dx in range(KW):
        for y in range(BAND_OUT):
            A[y : y + KH, dx * APAD + y] = K[:, dx]
    return A.astype(ml_dtypes.bfloat16)


def kernel(X, K, bias, _trace=False):
    global LAST_RESULTS
    X = np.asarray(X, dtype=np.float32)
    K = np.asarray(K, dtype=np.float32)
    bias_val = float(np.asarray(bias).reshape(-1)[0])

    A = _make_A(K)
    Xb = X.astype(ml_dtypes.bfloat16)

    in_maps = []
    for i in range(N_CORES):
        xm = Xb[MAIN_OUT * i : MAIN_OUT * i + MAIN_IN]  # contiguous view
        b = 64 + i // 2
        r0 = BAND_OUT * b
        rows = min(BAND_IN, H - r0)  # band 67 has only 18 real input rows
        xh = np.zeros((BAND_IN, HALF_IN_COLS), ml_dtypes.bfloat16)
        if i % 2 == 0:
            xh[:rows, :] = Xb[r0 : r0 + rows, 0:HALF_IN_COLS]
        else:
            xh[:rows, : W - 4096] = Xb[r0 : r0 + rows, 4096:W]
        in_maps.append({"Xm": xm, "Xh": xh, "A": A})

    nc = _build_nc(bias_val)
    res = run_bass_kernel_spmd(nc, in_maps, core_ids=list(range(N_CORES)), trace=_trace)
    LAST_RESULTS = res

    full = np.empty((OH, OW), np.float32)
    for i in range(N_CORES):
        full[MAIN_OUT * i : MAIN_OUT * (i + 1)] = res.results[i]["Om"].astype(
            np.float32
        )
        b = 64 + i // 2
        r0 = BAND_OUT * b
        nr = min(BAND_OUT, OH - r0)  # band 67: 12 valid rows
        oh = res.results[i]["Oh"].astype(np.float32)
        if i % 2 == 0:
            full[r0 : r0 + nr, 0:4096] = oh[:nr, :4096]
        else:
            full[r0 : r0 + nr, 4096:OW] = oh[:nr, : OW - 4096]
    return full


# revision 10
# speedup vs baseline: 1.0538x; 1.0538x over previous
"""Trainium2 Bass kernel: 7x7 valid 2D cross-correlation of an 8192x8192
fp32 image plus scalar bias, row-sharded across 8 NeuronCores.

Formulation (per core): the y-direction 7-tap convolution for a fixed kernel
column dx is a banded matmul: out_dx[y, x] = sum_r A_dx[r, y] * X[r, x] with
A_dx[r, y] = K[r - y, dx].  The full conv accumulates the 7 dx terms in PSUM
with the moving operand (image columns) shifted by dx.  Matmuls run in bf16
(inputs bf16, fp32 PSUM accumulate); the banded weight blocks are padded to
128 columns so the compiler's fast-weight-load path engages.

Work distribution: 8186 output rows = 68 bands of <=122 rows.  Each core gets
8 full bands (rows 976*i .. 976*i+976) plus HALF of one of bands 64..67
(8 column tiles), i.e. 136 (band, col-tile) units/core instead of 9 full
bands = 144 — the PE-time quantum is a 512-column matmul pass, so the old
layout wasted 8 units/core on a mostly-empty 9th band.  The half-band is
processed FIRST: its input is only ~1 MB, so the PE starts as soon as the
DMA rings come up instead of waiting for a full 2.1 MB slab.  Output is
stored per 1024-column pair tile immediately after its PSUM drain, so the
kernel tail after the last matmul is one small store, not a 2 MB band store.
"""

import numpy as np
import ml_dtypes

import concourse.bass as bass
import concourse.mybir as mybir
from concourse.tile import TileContext
from concourse.bass_utils import run_bass_kernel_spmd

H = W = 8192
KH = KW = 7
OH = OW = H - KH + 1          # 8186
N_CORES = 8
BAND_IN = 128                 # input rows per matmul band (partition dim)
BAND_OUT = BAND_IN - KH + 1   # 122 output rows per band
APAD = 128                    # A block columns (padded from BAND_OUT for FWL)
COL_TILE = 512                # moving-operand free dim (one PSUM bank, fp32)
F32 = mybir.dt.float32
BF16 = mybir.dt.bfloat16

MAIN_BANDS = 8                # full bands per core
MAIN_OUT = MAIN_BANDS * BAND_OUT      # 976
MAIN_IN = MAIN_OUT + KH - 1           # 982
HALF_TILES = 8                # col tiles in the half band
HALF_OUT_COLS = HALF_TILES * COL_TILE # 4096
HALF_IN_COLS = HALF_OUT_COLS + 8      # 4104 (6-col halo, padded to 8)

# Results object of the most recent hardware run (for test harnesses).
LAST_RESULTS = None


def _split_multi_waits(nc):
    """Walrus in this toolchain accepts at most ONE sync-wait per
    instruction; Tile's scheduler may attach several.  Hoist the extras onto
    single-wait InstEventSemaphore instructions inserted just before, on the
    same engine stream (a sequence of waits = AND of the conditions)."""
    uid = 0
    for fn in nc.m.functions:
        for blk in fn.blocks:
            new_list = []
            for inst in blk.instructions:
                si = getattr(inst, "sync_info", None)
                if si is not None and si.on_wait and len(si.on_wait) > 1:
                    waits = list(si.on_wait)
                    for w in waits[:-1]:
                        ev = mybir.InstEventSemaphore(
                            name=f"wait_split_{uid}",
                            ins=[],
                            outs=[],
                            sync_info=mybir.SyncInfo(on_wait=[w], on_update=[]),
                        )
                        uid += 1
                        ev.engine = inst.engine
                        new_list.append(ev)
                    si.on_wait = [waits[-1]]
                new_list.append(inst)
            blk.instructions[:] = new_list


def _build_nc(bias_val):
    nc = bass.Bass()
    Xm = nc.declare_dram_parameter("Xm", [MAIN_IN, W], BF16, isOutput=False)
    Xh = nc.declare_dram_parameter("Xh", [BAND_IN, HALF_IN_COLS], BF16, isOutput=False)
    A = nc.declare_dram_parameter("A", [BAND_IN, KW * APAD], BF16, isOutput=False)
    Om = nc.declare_dram_parameter("Om", [MAIN_OUT, OW], BF16, isOutput=True)
    Oh = nc.declare_dram_parameter("Oh", [BAND_OUT, HALF_OUT_COLS], BF16, isOutput=True)

    with TileContext(nc) as tc:
        with (
            tc.tile_pool(name="const", bufs=1) as cpool,
            tc.tile_pool(name="hx", bufs=1) as hxpool,
            tc.tile_pool(name="x", bufs=4) as xpool,
            tc.tile_pool(name="o", bufs=3) as opool,
            tc.tile_pool(name="ps", bufs=8, space="PSUM") as pspool,
        ):
            # Startup: split the small gating loads (A, half-band input)
            # across the three DMA queues so the PE starts on the half band
            # while the first full band is still streaming in.
            a_tile = cpool.tile([BAND_IN, KW * APAD], BF16)
            nc.sync.dma_start(out=a_tile[0:48, :], in_=A[0:48, :])
            nc.scalar.dma_start(out=a_tile[48:96, :], in_=A[48:96, :])
            nc.gpsimd.dma_start(out=a_tile[96:128, :], in_=A[96:128, :])

            # One half-band input tile (one descriptor per partition row);
            # partition-split across queues for parallel ramp.
            hx = hxpool.tile([BAND_IN, HALF_IN_COLS], BF16, tag="hx")
            nc.sync.dma_start(out=hx[0:40, :], in_=Xh[0:40, :])
            nc.scalar.dma_start(out=hx[40:80, :], in_=Xh[40:80, :])
            nc.gpsimd.dma_start(out=hx[80:128, :], in_=Xh[80:128, :])

            x_tiles = {}

            def issue_load(bi, split3=False):
                if bi >= MAIN_BANDS:
                    return
                r0 = bi * BAND_OUT
                xt = xpool.tile([BAND_IN, W], BF16, tag="x")
                if split3:
                    nc.gpsimd.dma_start(out=xt[0:64, :], in_=Xm[r0 : r0 + 64, :])
                    nc.sync.dma_start(out=xt[64:96, :], in_=Xm[r0 + 64 : r0 + 96, :])
                    nc.scalar.dma_start(out=xt[96:128, :], in_=Xm[r0 + 96 : r0 + 128, :])
                else:
                    nc.gpsimd.dma_start(out=xt[0:64, :], in_=Xm[r0 : r0 + 64, :])
                    nc.gpsimd.dma_start(out=xt[64:128, :], in_=Xm[r0 + 64 : r0 + 128, :])
                x_tiles[bi] = xt

            issue_load(0, split3=True)
            issue_load(1)
            issue_load(2)

            def conv_tile(x_tile, x0, w, o_tile, c0):
                """7 accumulating matmuls into a PSUM bank, drain to o_tile."""
                ps = pspool.tile([APAD, COL_TILE], F32)
                for dx in range(KW):
                    nc.tensor.matmul(
                        ps[:, :w],
                        lhsT=a_tile[:, dx * APAD : (dx + 1) * APAD],
                        rhs=x_tile[:, x0 + dx : x0 + dx + w],
                        start=(dx == 0),
                        stop=(dx == KW - 1),
                    )
                nc.vector.tensor_scalar_add(
                    o_tile[:, c0 : c0 + w], ps[:BAND_OUT, :w], float(bias_val)
                )

            # --- half band first (small input => earliest possible PE start).
            # Store rows mostly via gpsimd (SWDGE spreads descriptors; a fat
            # HWDGE DMA would serialize ~1.2us/row on one SDMA engine and
            # clog the queue behind it).
            o_half = opool.tile([BAND_OUT, HALF_OUT_COLS], BF16, tag="oh")
            for j in range(HALF_TILES):
                conv_tile(hx, j * COL_TILE, COL_TILE, o_half, j * COL_TILE)
            nc.sync.dma_start(out=Oh[0:16, :], in_=o_half[0:16, :])
            nc.gpsimd.dma_start(out=Oh[16:61, :], in_=o_half[16:61, :])
            nc.gpsimd.dma_start(out=Oh[61:106, :], in_=o_half[61:106, :])
            nc.scalar.dma_start(out=Oh[106:BAND_OUT, :], in_=o_half[106:BAND_OUT, :])

            # --- main bands: per-band o_tile stored as full-width row chunks
            # (one 16 KB descriptor per output row, the minimum possible).
            for bi in range(MAIN_BANDS):
                issue_load(bi + 3)
                x_tile = x_tiles.pop(bi)
                o_tile = opool.tile([BAND_OUT, OW], BF16, tag="om")
                for j in range(16):
                    x0 = j * COL_TILE
                    w = min(COL_TILE, OW - x0)
                    conv_tile(x_tile, x0, w, o_tile, x0)
                s = bi * BAND_OUT
                nchunks = 8
                bounds = [(BAND_OUT * k) // nchunks for k in range(nchunks + 1)]
                for k in range(nchunks):
                    p0, p1 = bounds[k], bounds[k + 1]
                    eng = (nc.gpsimd, nc.sync, nc.gpsimd, nc.scalar)[k % 4]
                    eng.dma_start(out=Om[s + p0 : s + p1, :], in_=o_tile[p0:p1, :])

    _split_multi_waits(nc)
    return nc


def _make_A(K):
    A = np.zeros((BAND_IN, KW * APAD), np.float32)
    for dx in range(KW):
        for y in range(BAND_OUT):
            A[y : y + KH, dx * APAD + y] = K[:, dx]
    return A.astype(ml_dtypes.bfloat16)


def kernel(X, K, bias, _trace=False):
    global LAST_RESULTS
    X = np.asarray(X, dtype=np.float32)
    K = np.asarray(K, dtype=np.float32)
    bias_val = float(np.asarray(bias).reshape(-1)[0])

    A = _make_A(K)
    Xb = X.astype(ml_dtypes.bfloat16)

    in_maps = []
    for i in range(N_CORES):
        xm = Xb[MAIN_OUT * i : MAIN_OUT * i + MAIN_IN]  # contiguous view
        b = 64 + i // 2
        r0 = BAND_OUT * b
        rows = min(BAND_IN, H - r0)  # band 67 has only 18 real input rows
        xh = np.zeros((BAND_IN, HALF_IN_COLS), ml_dtypes.bfloat16)
        if i % 2 == 0:
            xh[:rows, :] = Xb[r0 : r0 + rows, 0:HALF_IN_COLS]
        else:
            xh[:rows, : W - 4096] = Xb[r0 : r0 + rows, 4096:W]
        in_maps.append({"Xm": xm, "Xh": xh, "A": A})

    nc = _build_nc(bias_val)
    res = run_bass_kernel_spmd(nc, in_maps, core_ids=list(range(N_CORES)), trace=_trace)
    LAST_RESULTS = res

    full = np.empty((OH, OW), np.float32)
    for i in range(N_CORES):
        full[MAIN_OUT * i : MAIN_OUT * (i + 1)] = res.results[i]["Om"].astype(
            np.float32
        )
        b = 64 + i // 2
        r0 = BAND_OUT * b
        nr = min(BAND_OUT, OH - r0)  # band 67: 12 valid rows
        oh = res.results[i]["Oh"].astype(np.float32)
        if i % 2 == 0:
            full[r0 : r0 + nr, 0:4096] = oh[:nr, :4096]
        else:
            full[r0 : r0 + nr, 4096:OW] = oh[:nr, : OW - 4096]
    return full


# revision 11
# speedup vs baseline: 1.0592x; 1.0051x over previous
"""Trainium2 Bass kernel: 7x7 valid 2D cross-correlation of an 8192x8192
fp32 image plus scalar bias, row-sharded across 8 NeuronCores.

Formulation (per core): the y-direction 7-tap convolution for a fixed kernel
column dx is a banded matmul: out_dx[y, x] = sum_r A_dx[r, y] * X[r, x] with
A_dx[r, y] = K[r - y, dx].  The full conv accumulates the 7 dx terms in PSUM
with the moving operand (image columns) shifted by dx.  Matmuls run in bf16
(inputs bf16, fp32 PSUM accumulate); the banded weight blocks are padded to
128 columns so the compiler's fast-weight-load path engages.

Work distribution: 8186 output rows = 68 bands of <=122 rows.  Each core gets
8 full bands (rows 976*i .. 976*i+976) plus HALF of one of bands 64..67
(8 column tiles), i.e. 136 (band, col-tile) units/core instead of 9 full
bands = 144 — the PE-time quantum is a 512-column matmul pass, so the old
layout wasted 8 units/core on a mostly-empty 9th band.  The half-band is
processed FIRST: its input is only ~1 MB, so the PE starts as soon as the
DMA rings come up instead of waiting for a full 2.1 MB slab.  Output is
stored per 1024-column pair tile immediately after its PSUM drain, so the
kernel tail after the last matmul is one small store, not a 2 MB band store.
"""

import numpy as np
import ml_dtypes

import concourse.bass as bass
import concourse.mybir as mybir
from concourse.tile import TileContext
from concourse.bass_utils import run_bass_kernel_spmd

H = W = 8192
KH = KW = 7
OH = OW = H - KH + 1          # 8186
N_CORES = 8
BAND_IN = 128                 # input rows per matmul band (partition dim)
BAND_OUT = BAND_IN - KH + 1   # 122 output rows per band
APAD = 128                    # A block columns (padded from BAND_OUT for FWL)
COL_TILE = 512                # moving-operand free dim (one PSUM bank, fp32)
F32 = mybir.dt.float32
BF16 = mybir.dt.bfloat16

MAIN_BANDS = 8                # full bands per core
MAIN_OUT = MAIN_BANDS * BAND_OUT      # 976
MAIN_IN = MAIN_OUT + KH - 1           # 982
HALF_TILES = 8                # col tiles in the half band
HALF_OUT_COLS = HALF_TILES * COL_TILE # 4096
HALF_IN_COLS = HALF_OUT_COLS + 8      # 4104 (6-col halo, padded to 8)

# Results object of the most recent hardware run (for test harnesses).
LAST_RESULTS = None


def _split_multi_waits(nc):
    """Walrus in this toolchain accepts at most ONE sync-wait per
    instruction; Tile's scheduler may attach several.  Hoist the extras onto
    single-wait InstEventSemaphore instructions inserted just before, on the
    same engine stream (a sequence of waits = AND of the conditions)."""
    uid = 0
    for fn in nc.m.functions:
        for blk in fn.blocks:
            new_list = []
            for inst in blk.instructions:
                si = getattr(inst, "sync_info", None)
                if si is not None and si.on_wait and len(si.on_wait) > 1:
                    waits = list(si.on_wait)
                    for w in waits[:-1]:
                        ev = mybir.InstEventSemaphore(
                            name=f"wait_split_{uid}",
                            ins=[],
                            outs=[],
                            sync_info=mybir.SyncInfo(on_wait=[w], on_update=[]),
                        )
                        uid += 1
                        ev.engine = inst.engine
                        new_list.append(ev)
                    si.on_wait = [waits[-1]]
                new_list.append(inst)
            blk.instructions[:] = new_list


def _build_nc(bias_val):
    nc = bass.Bass()
    Xm = nc.declare_dram_parameter("Xm", [MAIN_IN, W], BF16, isOutput=False)
    Xh = nc.declare_dram_parameter("Xh", [BAND_IN, HALF_IN_COLS], BF16, isOutput=False)
    A = nc.declare_dram_parameter("A", [BAND_IN, KW * APAD], BF16, isOutput=False)
    Om = nc.declare_dram_parameter("Om", [MAIN_OUT, OW], BF16, isOutput=True)
    Oh = nc.declare_dram_parameter("Oh", [BAND_OUT, HALF_OUT_COLS], BF16, isOutput=True)

    with TileContext(nc) as tc:
        with (
            tc.tile_pool(name="const", bufs=1) as cpool,
            tc.tile_pool(name="hx", bufs=1) as hxpool,
            tc.tile_pool(name="x", bufs=4) as xpool,
            tc.tile_pool(name="o", bufs=3) as opool,
            tc.tile_pool(name="ps", bufs=8, space="PSUM") as pspool,
        ):
            # DMA rings serve strictly in order and each entry's wait gates
            # the ring (head-of-line).  Ring capacity is plentiful (~300 GB/s
            # aggregate burst) so the plan is about ISSUE ORDER: tiny gating
            # loads first, loads kept on the gpsimd ring, stores mostly on the
            # sync/scalar rings where their drain-waits can't block loads.
            a_tile = cpool.tile([BAND_IN, KW * APAD], BF16)
            nc.sync.dma_start(out=a_tile[0:48, :], in_=A[0:48, :])
            nc.scalar.dma_start(out=a_tile[48:96, :], in_=A[48:96, :])
            nc.gpsimd.dma_start(out=a_tile[96:128, :], in_=A[96:128, :])

            # Half-band input split: hx_a gates the 4 opening col tiles,
            # hx_b the 4 closing ones (processed at the very end).
            hx_a = hxpool.tile([BAND_IN, 2056], BF16, tag="hxa")
            hx_b = hxpool.tile([BAND_IN, HALF_IN_COLS - 2048], BF16, tag="hxb")
            nc.sync.dma_start(out=hx_a[0:64, :], in_=Xh[0:64, 0:2056])
            nc.scalar.dma_start(out=hx_a[64:128, :], in_=Xh[64:128, 0:2056])
            nc.gpsimd.dma_start(out=hx_b[0:64, :], in_=Xh[0:64, 2048:HALF_IN_COLS])
            nc.gpsimd.dma_start(out=hx_b[64:128, :], in_=Xh[64:128, 2048:HALF_IN_COLS])

            x_tiles = {}

            def issue_load(bi, split3=False):
                if bi >= MAIN_BANDS:
                    return
                r0 = bi * BAND_OUT
                xt = xpool.tile([BAND_IN, W], BF16, tag="x")
                if split3:
                    nc.gpsimd.dma_start(out=xt[0:64, :], in_=Xm[r0 : r0 + 64, :])
                    nc.sync.dma_start(out=xt[64:96, :], in_=Xm[r0 + 64 : r0 + 96, :])
                    nc.scalar.dma_start(out=xt[96:128, :], in_=Xm[r0 + 96 : r0 + 128, :])
                else:
                    nc.gpsimd.dma_start(out=xt[0:64, :], in_=Xm[r0 : r0 + 64, :])
                    nc.gpsimd.dma_start(out=xt[64:128, :], in_=Xm[r0 + 64 : r0 + 128, :])
                x_tiles[bi] = xt

            issue_load(0, split3=True)
            issue_load(1)
            issue_load(2)

            def conv_tile(x_tile, x0, w, o_tile, c0):
                """7 accumulating matmuls into a PSUM bank, drain to o_tile."""
                ps = pspool.tile([APAD, COL_TILE], F32)
                for dx in range(KW):
                    nc.tensor.matmul(
                        ps[:, :w],
                        lhsT=a_tile[:, dx * APAD : (dx + 1) * APAD],
                        rhs=x_tile[:, x0 + dx : x0 + dx + w],
                        start=(dx == 0),
                        stop=(dx == KW - 1),
                    )
                nc.vector.tensor_scalar_add(
                    o_tile[:, c0 : c0 + w], ps[:BAND_OUT, :w], float(bias_val)
                )

            # --- opening quarter band: 4 col tiles gated only on A + hx_a
            # (~0.7 MB), so the PE starts within a few microseconds.
            o_ha = opool.tile([BAND_OUT, 4 * COL_TILE], BF16, tag="oha")
            for j in range(4):
                conv_tile(hx_a, j * COL_TILE, COL_TILE, o_ha, j * COL_TILE)
            nc.sync.dma_start(out=Oh[0:31, 0:2048], in_=o_ha[0:31, :])
            nc.scalar.dma_start(out=Oh[31:61, 0:2048], in_=o_ha[31:61, :])
            nc.sync.dma_start(out=Oh[61:92, 0:2048], in_=o_ha[61:92, :])
            nc.scalar.dma_start(out=Oh[92:BAND_OUT, 0:2048], in_=o_ha[92:BAND_OUT, :])

            # --- main bands: loads stay on the gpsimd ring; stores weighted
            # onto sync/scalar so their drain-waits never block loads.
            store_engs = (nc.sync, nc.scalar, nc.gpsimd, nc.sync, nc.scalar, nc.gpsimd, nc.sync, nc.scalar)
            for bi in range(MAIN_BANDS):
                issue_load(bi + 3)
                x_tile = x_tiles.pop(bi)
                o_tile = opool.tile([BAND_OUT, OW], BF16, tag="om")
                for j in range(16):
                    x0 = j * COL_TILE
                    w = min(COL_TILE, OW - x0)
                    conv_tile(x_tile, x0, w, o_tile, x0)
                s = bi * BAND_OUT
                bounds = [(BAND_OUT * k) // 8 for k in range(9)]
                for k in range(8):
                    p0, p1 = bounds[k], bounds[k + 1]
                    store_engs[k].dma_start(
                        out=Om[s + p0 : s + p1, :], in_=o_tile[p0:p1, :]
                    )

            # --- closing quarter band: its input has been resident since
            # startup; the tail after the last matmul is only a 0.5 MB store.
            o_hb = opool.tile([BAND_OUT, 4 * COL_TILE], BF16, tag="ohb")
            for j in range(4, HALF_TILES):
                conv_tile(hx_b, j * COL_TILE - 2048, COL_TILE, o_hb, (j - 4) * COL_TILE)
            nc.sync.dma_start(out=Oh[0:31, 2048:HALF_OUT_COLS], in_=o_hb[0:31, :])
            nc.scalar.dma_start(out=Oh[31:61, 2048:HALF_OUT_COLS], in_=o_hb[31:61, :])
            nc.gpsimd.dma_start(out=Oh[61:92, 2048:HALF_OUT_COLS], in_=o_hb[61:92, :])
            nc.gpsimd.dma_start(out=Oh[92:BAND_OUT, 2048:HALF_OUT_COLS], in_=o_hb[92:BAND_OUT, :])

    _split_multi_waits(nc)
    return nc


def _make_A(K):
    A = np.zeros((BAND_IN, KW * APAD), np.float32)
    for dx in range(KW):
        for y in range(BAND_OUT):
            A[y : y + KH, dx * APAD + y] = K[:, dx]
    return A.astype(ml_dtypes.bfloat16)


def kernel(X, K, bias, _trace=False):
    global LAST_RESULTS
    X = np.asarray(X, dtype=np.float32)
    K = np.asarray(K, dtype=np.float32)
    bias_val = float(np.asarray(bias).reshape(-1)[0])

    A = _make_A(K)
    Xb = X.astype(ml_dtypes.bfloat16)

    in_maps = []
    for i in range(N_CORES):
        xm = Xb[MAIN_OUT * i : MAIN_OUT * i + MAIN_IN]  # contiguous view
        b = 64 + i // 2
        r0 = BAND_OUT * b
        rows = min(BAND_IN, H - r0)  # band 67 has only 18 real input rows
        xh = np.zeros((BAND_IN, HALF_IN_COLS), ml_dtypes.bfloat16)
        if i % 2 == 0:
            xh[:rows, :] = Xb[r0 : r0 + rows, 0:HALF_IN_COLS]
        else:
            xh[:rows, : W - 4096] = Xb[r0 : r0 + rows, 4096:W]
        in_maps.append({"Xm": xm, "Xh": xh, "A": A})

    nc = _build_nc(bias_val)
    res = run_bass_kernel_spmd(nc, in_maps, core_ids=list(range(N_CORES)), trace=_trace)
    LAST_RESULTS = res

    full = np.empty((OH, OW), np.float32)
    for i in range(N_CORES):
        full[MAIN_OUT * i : MAIN_OUT * (i + 1)] = res.results[i]["Om"].astype(
            np.float32
        )
        b = 64 + i // 2
        r0 = BAND_OUT * b
        nr = min(BAND_OUT, OH - r0)  # band 67: 12 valid rows
        oh = res.results[i]["Oh"].astype(np.float32)
        if i % 2 == 0:
            full[r0 : r0 + nr, 0:4096] = oh[:nr, :4096]
        else:
            full[r0 : r0 + nr, 4096:OW] = oh[:nr, : OW - 4096]
    return full


# revision 12
# speedup vs baseline: 1.0721x; 1.0121x over previous
"""Trainium2 Bass kernel: 7x7 valid 2D cross-correlation of an 8192x8192
fp32 image plus scalar bias, row-sharded across 8 NeuronCores.

Formulation (per core): the y-direction 7-tap convolution for a fixed kernel
column dx is a banded matmul: out_dx[y, x] = sum_r A_dx[r, y] * X[r, x] with
A_dx[r, y] = K[r - y, dx].  The full conv accumulates the 7 dx terms in PSUM
with the moving operand (image columns) shifted by dx.  Matmuls run in bf16
(inputs bf16, fp32 PSUM accumulate); the banded weight blocks are padded to
128 columns so the compiler's fast-weight-load path engages.

Work distribution: 8186 output rows = 68 bands of <=122 rows.  Each core gets
8 full bands (rows 976*i .. 976*i+976) plus HALF of one of bands 64..67
(8 column tiles), i.e. 136 (band, col-tile) units/core instead of 9 full
bands = 144 — the PE-time quantum is a 512-column matmul pass, so the old
layout wasted 8 units/core on a mostly-empty 9th band.  The half-band is
processed FIRST: its input is only ~1 MB, so the PE starts as soon as the
DMA rings come up instead of waiting for a full 2.1 MB slab.  Output is
stored per 1024-column pair tile immediately after its PSUM drain, so the
kernel tail after the last matmul is one small store, not a 2 MB band store.
"""

import numpy as np
import ml_dtypes

import concourse.bass as bass
import concourse.mybir as mybir
from concourse.tile import TileContext
from concourse.bass_utils import run_bass_kernel_spmd

H = W = 8192
KH = KW = 7
OH = OW = H - KH + 1          # 8186
N_CORES = 8
BAND_IN = 128                 # input rows per matmul band (partition dim)
BAND_OUT = BAND_IN - KH + 1   # 122 output rows per band
APAD = 128                    # A block columns (padded from BAND_OUT for FWL)
COL_TILE = 512                # moving-operand free dim (one PSUM bank, fp32)
F32 = mybir.dt.float32
BF16 = mybir.dt.bfloat16

MAIN_BANDS = 8                # full bands per core
MAIN_OUT = MAIN_BANDS * BAND_OUT      # 976
MAIN_IN = MAIN_OUT + KH - 1           # 982
HALF_TILES = 8                # col tiles in the half band
HALF_OUT_COLS = HALF_TILES * COL_TILE # 4096
HALF_IN_COLS = HALF_OUT_COLS + 8      # 4104 (6-col halo, padded to 8)

# Results object of the most recent hardware run (for test harnesses).
LAST_RESULTS = None


def _split_multi_waits(nc):
    """Walrus in this toolchain accepts at most ONE sync-wait per
    instruction; Tile's scheduler may attach several.  Hoist the extras onto
    single-wait InstEventSemaphore instructions inserted just before, on the
    same engine stream (a sequence of waits = AND of the conditions)."""
    uid = 0
    for fn in nc.m.functions:
        for blk in fn.blocks:
            new_list = []
            for inst in blk.instructions:
                si = getattr(inst, "sync_info", None)
                if si is not None and si.on_wait and len(si.on_wait) > 1:
                    waits = list(si.on_wait)
                    for w in waits[:-1]:
                        ev = mybir.InstEventSemaphore(
                            name=f"wait_split_{uid}",
                            ins=[],
                            outs=[],
                            sync_info=mybir.SyncInfo(on_wait=[w], on_update=[]),
                        )
                        uid += 1
                        ev.engine = inst.engine
                        new_list.append(ev)
                    si.on_wait = [waits[-1]]
                new_list.append(inst)
            blk.instructions[:] = new_list


def _build_nc(bias_val):
    nc = bass.Bass()
    Xm = nc.declare_dram_parameter("Xm", [MAIN_IN, W], BF16, isOutput=False)
    Xh = nc.declare_dram_parameter("Xh", [BAND_IN, HALF_IN_COLS], BF16, isOutput=False)
    A = nc.declare_dram_parameter("A", [BAND_IN, KW * APAD], BF16, isOutput=False)
    Om = nc.declare_dram_parameter("Om", [MAIN_OUT, OW], BF16, isOutput=True)
    Oh = nc.declare_dram_parameter("Oh", [BAND_OUT, HALF_OUT_COLS], BF16, isOutput=True)

    with TileContext(nc) as tc:
        with (
            tc.tile_pool(name="const", bufs=1) as cpool,
            tc.tile_pool(name="hx", bufs=1) as hxpool,
            tc.tile_pool(name="x", bufs=4) as xpool,
            tc.tile_pool(name="o", bufs=3) as opool,
            tc.tile_pool(name="ps", bufs=8, space="PSUM") as pspool,
        ):
            # DMA rings serve strictly in order and each entry's wait gates
            # the ring (head-of-line).  Ring capacity is plentiful (~300 GB/s
            # aggregate burst) so the plan is about ISSUE ORDER: tiny gating
            # loads first, loads kept on the gpsimd ring, stores mostly on the
            # sync/scalar rings where their drain-waits can't block loads.
            # All gating loads ride the gpsimd (SWDGE) ring, which spreads a
            # single DMA's rows across all 16 SDMA engines; an HWDGE DMA
            # serializes ~0.6us/row on one engine and would stall the PE.
            a_tile = cpool.tile([BAND_IN, KW * APAD], BF16)
            nc.gpsimd.dma_start(out=a_tile[:, :], in_=A[:, :])

            # Half-band input split: hx_a gates the 4 opening col tiles,
            # hx_b the 4 closing ones (loaded later, used at the very end).
            hx_a = hxpool.tile([BAND_IN, 2056], BF16, tag="hxa")
            hx_b = hxpool.tile([BAND_IN, HALF_IN_COLS - 2048], BF16, tag="hxb")
            nc.gpsimd.dma_start(out=hx_a[:, :], in_=Xh[:, 0:2056])

            x_tiles = {}

            def issue_load(bi):
                if bi >= MAIN_BANDS:
                    return
                r0 = bi * BAND_OUT
                xt = xpool.tile([BAND_IN, W], BF16, tag="x")
                nc.gpsimd.dma_start(out=xt[0:64, :], in_=Xm[r0 : r0 + 64, :])
                nc.gpsimd.dma_start(out=xt[64:128, :], in_=Xm[r0 + 64 : r0 + 128, :])
                x_tiles[bi] = xt

            issue_load(0)
            issue_load(1)
            issue_load(2)

            def conv_tile(x_tile, x0, w, o_tile, c0):
                """7 accumulating matmuls into a PSUM bank, drain to o_tile."""
                ps = pspool.tile([APAD, COL_TILE], F32)
                for dx in range(KW):
                    nc.tensor.matmul(
                        ps[:, :w],
                        lhsT=a_tile[:, dx * APAD : (dx + 1) * APAD],
                        rhs=x_tile[:, x0 + dx : x0 + dx + w],
                        start=(dx == 0),
                        stop=(dx == KW - 1),
                    )
                nc.vector.tensor_scalar_add(
                    o_tile[:, c0 : c0 + w], ps[:BAND_OUT, :w], float(bias_val)
                )

            # --- opening quarter band: 4 col tiles gated only on A + hx_a
            # (~0.7 MB), so the PE starts within a few microseconds.
            o_ha = opool.tile([BAND_OUT, 4 * COL_TILE], BF16, tag="oha")
            for j in range(4):
                conv_tile(hx_a, j * COL_TILE, COL_TILE, o_ha, j * COL_TILE)
            nc.sync.dma_start(out=Oh[0:31, 0:2048], in_=o_ha[0:31, :])
            nc.scalar.dma_start(out=Oh[31:61, 0:2048], in_=o_ha[31:61, :])
            nc.sync.dma_start(out=Oh[61:92, 0:2048], in_=o_ha[61:92, :])
            nc.scalar.dma_start(out=Oh[92:BAND_OUT, 0:2048], in_=o_ha[92:BAND_OUT, :])

            # --- main bands: loads stay on the gpsimd ring; stores weighted
            # onto sync/scalar so their drain-waits never block loads.
            store_engs = (nc.sync, nc.scalar, nc.gpsimd, nc.sync, nc.scalar, nc.gpsimd, nc.sync, nc.scalar)
            for bi in range(MAIN_BANDS):
                issue_load(bi + 3)
                if bi == 0:
                    nc.gpsimd.dma_start(out=hx_b[:, :], in_=Xh[:, 2048:HALF_IN_COLS])
                x_tile = x_tiles.pop(bi)
                o_tile = opool.tile([BAND_OUT, OW], BF16, tag="om")
                for j in range(16):
                    x0 = j * COL_TILE
                    w = min(COL_TILE, OW - x0)
                    conv_tile(x_tile, x0, w, o_tile, x0)
                s = bi * BAND_OUT
                # last band: fine chunks across all rings so the flush after
                # the final matmul is bounded by ~6 rows of serial DMA, not 15
                nchunks = 21 if bi == MAIN_BANDS - 1 else 8
                bounds = [(BAND_OUT * k) // nchunks for k in range(nchunks + 1)]
                for k in range(nchunks):
                    p0, p1 = bounds[k], bounds[k + 1]
                    eng = store_engs[k % 8] if nchunks == 8 else (nc.sync, nc.scalar, nc.gpsimd)[k % 3]
                    eng.dma_start(out=Om[s + p0 : s + p1, :], in_=o_tile[p0:p1, :])

            # --- closing quarter band: its input has been resident since
            # startup; the tail after the last matmul is only a 0.5 MB store.
            o_hb = opool.tile([BAND_OUT, 4 * COL_TILE], BF16, tag="ohb")
            for j in range(4, HALF_TILES):
                conv_tile(hx_b, j * COL_TILE - 2048, COL_TILE, o_hb, (j - 4) * COL_TILE)
            bounds = [(BAND_OUT * k) // 12 for k in range(13)]
            for k in range(12):
                p0, p1 = bounds[k], bounds[k + 1]
                eng = (nc.sync, nc.scalar, nc.gpsimd)[k % 3]
                eng.dma_start(
                    out=Oh[p0:p1, 2048:HALF_OUT_COLS], in_=o_hb[p0:p1, :]
                )

    _split_multi_waits(nc)
    return nc


def _make_A(K):
    A = np.zeros((BAND_IN, KW * APAD), np.float32)
    for dx in range(KW):
        for y in range(BAND_OUT):
            A[y : y + KH, dx * APAD + y] = K[:, dx]
    return A.astype(ml_dtypes.bfloat16)


def kernel(X, K, bias, _trace=False):
    global LAST_RESULTS
    X = np.asarray(X, dtype=np.float32)
    K = np.asarray(K, dtype=np.float32)
    bias_val = float(np.asarray(bias).reshape(-1)[0])

    A = _make_A(K)
    Xb = X.astype(ml_dtypes.bfloat16)

    in_maps = []
    for i in range(N_CORES):
        xm = Xb[MAIN_OUT * i : MAIN_OUT * i + MAIN_IN]  # contiguous view
        b = 64 + i // 2
        r0 = BAND_OUT * b
        rows = min(BAND_IN, H - r0)  # band 67 has only 18 real input rows
        xh = np.zeros((BAND_IN, HALF_IN_COLS), ml_dtypes.bfloat16)
        if i % 2 == 0:
            xh[:rows, :] = Xb[r0 : r0 + rows, 0:HALF_IN_COLS]
        else:
            xh[:rows, : W - 4096] = Xb[r0 : r0 + rows, 4096:W]
        in_maps.append({"Xm": xm, "Xh": xh, "A": A})

    nc = _build_nc(bias_val)
    res = run_bass_kernel_spmd(nc, in_maps, core_ids=list(range(N_CORES)), trace=_trace)
    LAST_RESULTS = res

    full = np.empty((OH, OW), np.float32)
    for i in range(N_CORES):
        full[MAIN_OUT * i : MAIN_OUT * (i + 1)] = res.results[i]["Om"].astype(
            np.float32
        )
        b = 64 + i // 2
        r0 = BAND_OUT * b
        nr = min(BAND_OUT, OH - r0)  # band 67: 12 valid rows
        oh = res.results[i]["Oh"].astype(np.float32)
        if i % 2 == 0:
            full[r0 : r0 + nr, 0:4096] = oh[:nr, :4096]
        else:
            full[r0 : r0 + nr, 4096:OW] = oh[:nr, : OW - 4096]
    return full


# revision 13
# speedup vs baseline: 1.0934x; 1.0199x over previous
"""Trainium2 Bass kernel: 7x7 valid 2D cross-correlation of an 8192x8192
fp32 image plus scalar bias, row-sharded across 8 NeuronCores.

Formulation (per core): the y-direction 7-tap convolution for a fixed kernel
column dx is a banded matmul: out_dx[y, x] = sum_r A_dx[r, y] * X[r, x] with
A_dx[r, y] = K[r - y, dx].  The full conv accumulates the 7 dx terms in PSUM
with the moving operand (image columns) shifted by dx.  Matmuls run in bf16
(inputs bf16, fp32 PSUM accumulate); the banded weight blocks are padded to
128 columns so the compiler's fast-weight-load path engages.

Work distribution: 8186 output rows = 68 bands of <=122 rows.  Each core gets
8 full bands (rows 976*i .. 976*i+976) plus HALF of one of bands 64..67
(8 column tiles), i.e. 136 (band, col-tile) units/core instead of 9 full
bands = 144 — the PE-time quantum is a 512-column matmul pass, so the old
layout wasted 8 units/core on a mostly-empty 9th band.  The half-band is
processed FIRST: its input is only ~1 MB, so the PE starts as soon as the
DMA rings come up instead of waiting for a full 2.1 MB slab.  Output is
stored per 1024-column pair tile immediately after its PSUM drain, so the
kernel tail after the last matmul is one small store, not a 2 MB band store.
"""

import numpy as np
import ml_dtypes

import concourse.bass as bass
import concourse.mybir as mybir
from concourse.tile import TileContext
from concourse.bass_utils import run_bass_kernel_spmd

H = W = 8192
KH = KW = 7
OH = OW = H - KH + 1          # 8186
N_CORES = 8
BAND_IN = 128                 # input rows per matmul band (partition dim)
BAND_OUT = BAND_IN - KH + 1   # 122 output rows per band
APAD = 128                    # A block columns (padded from BAND_OUT for FWL)
COL_TILE = 512                # moving-operand free dim (one PSUM bank, fp32)
F32 = mybir.dt.float32
BF16 = mybir.dt.bfloat16

MAIN_BANDS = 8                # full bands per core
MAIN_OUT = MAIN_BANDS * BAND_OUT      # 976
MAIN_IN = MAIN_OUT + KH - 1           # 982
HALF_TILES = 8                # col tiles in the half band
HALF_OUT_COLS = HALF_TILES * COL_TILE # 4096
HALF_IN_COLS = HALF_OUT_COLS + 8      # 4104 (6-col halo, padded to 8)

# Results object of the most recent hardware run (for test harnesses).
LAST_RESULTS = None


def _split_multi_waits(nc):
    """Walrus in this toolchain accepts at most ONE sync-wait per
    instruction; Tile's scheduler may attach several.  Hoist the extras onto
    single-wait InstEventSemaphore instructions inserted just before, on the
    same engine stream (a sequence of waits = AND of the conditions)."""
    uid = 0
    for fn in nc.m.functions:
        for blk in fn.blocks:
            new_list = []
            for inst in blk.instructions:
                si = getattr(inst, "sync_info", None)
                if si is not None and si.on_wait and len(si.on_wait) > 1:
                    waits = list(si.on_wait)
                    for w in waits[:-1]:
                        ev = mybir.InstEventSemaphore(
                            name=f"wait_split_{uid}",
                            ins=[],
                            outs=[],
                            sync_info=mybir.SyncInfo(on_wait=[w], on_update=[]),
                        )
                        uid += 1
                        ev.engine = inst.engine
                        new_list.append(ev)
                    si.on_wait = [waits[-1]]
                new_list.append(inst)
            blk.instructions[:] = new_list


def _build_nc(bias_val):
    nc = bass.Bass()
    Xm = nc.declare_dram_parameter("Xm", [MAIN_IN, W], BF16, isOutput=False)
    Xh = nc.declare_dram_parameter("Xh", [BAND_IN, HALF_IN_COLS], BF16, isOutput=False)
    A = nc.declare_dram_parameter("A", [BAND_IN, KW * APAD], BF16, isOutput=False)
    Om = nc.declare_dram_parameter("Om", [MAIN_OUT, OW], BF16, isOutput=True)
    Oh = nc.declare_dram_parameter("Oh", [BAND_OUT, HALF_OUT_COLS], BF16, isOutput=True)

    with TileContext(nc) as tc:
        with (
            tc.tile_pool(name="const", bufs=1) as cpool,
            tc.tile_pool(name="hx", bufs=1) as hxpool,
            tc.tile_pool(name="x", bufs=4) as xpool,
            tc.tile_pool(name="o", bufs=3) as opool,
            tc.tile_pool(name="ps", bufs=8, space="PSUM") as pspool,
        ):
            # DMA rings serve strictly in order and each entry's wait gates
            # the ring (head-of-line).  Ring capacity is plentiful (~300 GB/s
            # aggregate burst) so the plan is about ISSUE ORDER: tiny gating
            # loads first, loads kept on the gpsimd ring, stores mostly on the
            # sync/scalar rings where their drain-waits can't block loads.
            # All gating loads ride the gpsimd (SWDGE) ring, which spreads a
            # single DMA's rows across all 16 SDMA engines; an HWDGE DMA
            # serializes ~0.6us/row on one engine and would stall the PE.
            a_tile = cpool.tile([BAND_IN, KW * APAD], BF16)
            nc.gpsimd.dma_start(out=a_tile[:, :], in_=A[:, :])

            # Half-band input split: hx_a gates the 4 opening col tiles,
            # hx_b the 4 closing ones (loaded later, used at the very end).
            hx_a = hxpool.tile([BAND_IN, 2056], BF16, tag="hxa")
            hx_b = hxpool.tile([BAND_IN, HALF_IN_COLS - 2048], BF16, tag="hxb")
            nc.gpsimd.dma_start(out=hx_a[:, :], in_=Xh[:, 0:2056])

            x_tiles = {}

            def issue_load(bi):
                if bi >= MAIN_BANDS:
                    return
                r0 = bi * BAND_OUT
                xt = xpool.tile([BAND_IN, W], BF16, tag="x")
                nc.gpsimd.dma_start(out=xt[0:64, :], in_=Xm[r0 : r0 + 64, :])
                nc.gpsimd.dma_start(out=xt[64:128, :], in_=Xm[r0 + 64 : r0 + 128, :])
                x_tiles[bi] = xt

            issue_load(0)
            issue_load(1)
            issue_load(2)

            def conv_tile(x_tile, x0, w, o_tile, c0):
                """7 accumulating matmuls into a PSUM bank, drain to o_tile."""
                ps = pspool.tile([APAD, COL_TILE], F32)
                for dx in range(KW):
                    nc.tensor.matmul(
                        ps[:, :w],
                        lhsT=a_tile[:, dx * APAD : (dx + 1) * APAD],
                        rhs=x_tile[:, x0 + dx : x0 + dx + w],
                        start=(dx == 0),
                        stop=(dx == KW - 1),
                    )
                nc.vector.tensor_scalar_add(
                    o_tile[:, c0 : c0 + w], ps[:BAND_OUT, :w], float(bias_val)
                )

            # --- opening quarter band: 4 col tiles gated only on A + hx_a
            # (~0.7 MB), so the PE starts within a few microseconds.
            o_ha = opool.tile([BAND_OUT, 4 * COL_TILE], BF16, tag="oha")
            for j in range(4):
                conv_tile(hx_a, j * COL_TILE, COL_TILE, o_ha, j * COL_TILE)
            nc.sync.dma_start(out=Oh[0:31, 0:2048], in_=o_ha[0:31, :])
            nc.scalar.dma_start(out=Oh[31:61, 0:2048], in_=o_ha[31:61, :])
            nc.sync.dma_start(out=Oh[61:92, 0:2048], in_=o_ha[61:92, :])
            nc.scalar.dma_start(out=Oh[92:BAND_OUT, 0:2048], in_=o_ha[92:BAND_OUT, :])

            # --- main bands: loads stay on the gpsimd ring; stores weighted
            # onto sync/scalar so their drain-waits never block loads.
            # The gpsimd (SWDGE) ring spreads each DMA's rows across all 16
            # SDMA engines (~250+ GB/s); the sync/scalar HWDGE rings serialize
            # ~0.6us/row (~26 GB/s each).  So gpsimd carries the bulk of the
            # stores too; HWDGE gets one 15-row chunk each per band.
            for bi in range(MAIN_BANDS):
                issue_load(bi + 3)
                if bi == 0:
                    nc.gpsimd.dma_start(out=hx_b[:, :], in_=Xh[:, 2048:HALF_IN_COLS])
                x_tile = x_tiles.pop(bi)
                o_tile = opool.tile([BAND_OUT, OW], BF16, tag="om")
                for j in range(16):
                    x0 = j * COL_TILE
                    w = min(COL_TILE, OW - x0)
                    conv_tile(x_tile, x0, w, o_tile, x0)
                s = bi * BAND_OUT
                if bi < MAIN_BANDS - 1:
                    nc.gpsimd.dma_start(out=Om[s : s + 46, :], in_=o_tile[0:46, :])
                    nc.gpsimd.dma_start(out=Om[s + 46 : s + 92, :], in_=o_tile[46:92, :])
                    nc.sync.dma_start(out=Om[s + 92 : s + 107, :], in_=o_tile[92:107, :])
                    nc.scalar.dma_start(out=Om[s + 107 : s + BAND_OUT, :], in_=o_tile[107:BAND_OUT, :])
                else:
                    # final band: gpsimd-only in staggered chunks + tiny HWDGE
                    # slices => flush is a few us, not an HWDGE row-serial crawl
                    for p0, p1 in ((0, 30), (30, 60), (60, 90), (90, 110)):
                        nc.gpsimd.dma_start(out=Om[s + p0 : s + p1, :], in_=o_tile[p0:p1, :])
                    nc.sync.dma_start(out=Om[s + 110 : s + 116, :], in_=o_tile[110:116, :])
                    nc.scalar.dma_start(out=Om[s + 116 : s + BAND_OUT, :], in_=o_tile[116:BAND_OUT, :])

            # --- closing quarter band: its input has been resident since
            # startup; the tail after the last matmul is only a 0.5 MB store.
            o_hb = opool.tile([BAND_OUT, 4 * COL_TILE], BF16, tag="ohb")
            for j in range(4, HALF_TILES):
                conv_tile(hx_b, j * COL_TILE - 2048, COL_TILE, o_hb, (j - 4) * COL_TILE)
            for p0, p1 in ((0, 40), (40, 80), (80, 110)):
                nc.gpsimd.dma_start(out=Oh[p0:p1, 2048:HALF_OUT_COLS], in_=o_hb[p0:p1, :])
            nc.sync.dma_start(out=Oh[110:116, 2048:HALF_OUT_COLS], in_=o_hb[110:116, :])
            nc.scalar.dma_start(out=Oh[116:BAND_OUT, 2048:HALF_OUT_COLS], in_=o_hb[116:BAND_OUT, :])

    _split_multi_waits(nc)
    return nc


def _make_A(K):
    A = np.zeros((BAND_IN, KW * APAD), np.float32)
    for dx in range(KW):
        for y in range(BAND_OUT):
            A[y : y + KH, dx * APAD + y] = K[:, dx]
    return A.astype(ml_dtypes.bfloat16)


def kernel(X, K, bias, _trace=False):
    global LAST_RESULTS
    X = np.asarray(X, dtype=np.float32)
    K = np.asarray(K, dtype=np.float32)
    bias_val = float(np.asarray(bias).reshape(-1)[0])

    A = _make_A(K)
    Xb = X.astype(ml_dtypes.bfloat16)

    in_maps = []
    for i in range(N_CORES):
        xm = Xb[MAIN_OUT * i : MAIN_OUT * i + MAIN_IN]  # contiguous view
        b = 64 + i // 2
        r0 = BAND_OUT * b
        rows = min(BAND_IN, H - r0)  # band 67 has only 18 real input rows
        xh = np.zeros((BAND_IN, HALF_IN_COLS), ml_dtypes.bfloat16)
        if i % 2 == 0:
            xh[:rows, :] = Xb[r0 : r0 + rows, 0:HALF_IN_COLS]
        else:
            xh[:rows, : W - 4096] = Xb[r0 : r0 + rows, 4096:W]
        in_maps.append({"Xm": xm, "Xh": xh, "A": A})

    nc = _build_nc(bias_val)
    res = run_bass_kernel_spmd(nc, in_maps, core_ids=list(range(N_CORES)), trace=_trace)
    LAST_RESULTS = res

    full = np.empty((OH, OW), np.float32)
    for i in range(N_CORES):
        full[MAIN_OUT * i : MAIN_OUT * (i + 1)] = res.results[i]["Om"].astype(
            np.float32
        )
        b = 64 + i // 2
        r0 = BAND_OUT * b
        nr = min(BAND_OUT, OH - r0)  # band 67: 12 valid rows
        oh = res.results[i]["Oh"].astype(np.float32)
        if i % 2 == 0:
            full[r0 : r0 + nr, 0:4096] = oh[:nr, :4096]
        else:
            full[r0 : r0 + nr, 4096:OW] = oh[:nr, : OW - 4096]
    return full
